# revision 4
# baseline (speedup 1.0000x reference)
"""Multi-head attention (qkv pointwise-conv projection + softmax attention)
on 8 Trainium2 NeuronCores.

Problem shapes (hardcoded):
    x:     [B=4, D=512, L=2048] f32
    w_qkv: [3*D=1536, D=512]    f32
    out:   [B, D, L]            f32

Sharding: 2 cores per batch element; each core owns 4 of the 8 heads
(tensor-parallel on the qkv output channels). Core c -> batch c//2,
head group c%2 (heads 4*(c%2) .. 4*(c%2)+3).

Design (v3, trace-driven): the kernel is bound by the ScalarE exp floor
(128 activations x [128,1024] ~ 1.11us each = 142us; exp exists only on
ScalarE - walrus rejects Pool-engine activations, and GPSIMD cannot touch
PSUM). Everything else is scheduled to keep ScalarE 100% busy:
  - software-pipelined period: st(t+1) emitted first, attn@v lagged one
    period (se(t-1)), projection fillers chopped to <=~350ns pieces so no
    period's PE work exceeds the 1112ns exp cadence
  - separate PSUM tag rings (st 2x2 banks, o_acc 2x1, proj/bcast 2x1) so
    transient proj tiles never wait on live accumulators
  - input DMA split across the sync/scalar/vector HWDGE rings with the
    critical slices (wqk pair-0, x chunk 0) first
  - softmax normalization via PE ones-broadcast of 1/den (no DRAM bounce)

Per-core kernel (bf16 compute, f32 accumulate):
    Q/K proj:  q[o,l] = sum_d w[o,d] x[d,l]   (layout [head_dim, L])
    V proj  :  vT[l,o] stored per head with a fused ones-column so
               attn@[v|1] yields values + softmax denominator
    scores  :  St[j,i] = sum_d k[d,j] q[d,i]  (two heads packed in the
               128-row PE array via row tiling; the pair streams
               concurrently)
    softmax :  exp on ScalarE (scale folded in), no max subtraction
               (scores are O(1) by construction)
    attn@v  :  O[d(+den),i] accumulated over j blocks in PSUM
    norm    :  O[d,i] * bcast(1/den[i]) via ones-matmul broadcast
"""

import os
import numpy as np

B, D, L, H = 4, 512, 2048, 8
HD = D // H  # 64
N_CORES = 8
SCALE = float(D) ** -0.5

# module-level knobs for test.py; harness uses defaults
TRACE = False
LAST_RESULTS = None

_COMPILED = {}


def _build_nc():
    from contextlib import ExitStack

    import concourse.bass as bass
    import concourse.mybir as mybir
    import concourse.tile as tile
    from concourse.bacc import Bacc

    F32 = mybir.dt.float32
    BF16 = mybir.dt.bfloat16
    Exp = mybir.ActivationFunctionType.Exp

    # Bacc (not plain Bass): its finalize() runs the legalization passes that
    # split multi-wait matmuls (walrus MM struct supports only 1 sync wait).
    nc = Bacc("TRN2", target_bir_lowering=False, debug=False)
    # host pre-permuted layouts -> fully contiguous DMA descriptors
    # x: [p, lc, dc, l'] where d = dc*128+p, l = lc*512+l'
    x_d = nc.dram_tensor("x", [128, 4, 4, 512], BF16, kind="ExternalInput")
    # wT pair-major: [p, pair, dc, (q128|k128)] so the pair-0 slice (needed
    # for the first projections) is one contiguous DMA
    wqk_d = nc.dram_tensor("wqkT", [128, 2, 4, 256], BF16, kind="ExternalInput")
    wv_d = nc.dram_tensor("wvT", [128, 4, 256], BF16, kind="ExternalInput")
    out_d = nc.dram_tensor("out", [256, L], F32, kind="ExternalOutput")

    NJB = L // 128  # 16 key blocks
    NIC = L // 512  # 4 query chunks

    with ExitStack() as ctx:
        tc = ctx.enter_context(tile.TileContext(nc))
        const = ctx.enter_context(tc.tile_pool(name="const", bufs=1))
        qkp = ctx.enter_context(tc.tile_pool(name="qkp", bufs=1))
        sx = ctx.enter_context(tc.tile_pool(name="sx", bufs=4))
        nrm = ctx.enter_context(tc.tile_pool(name="nrm", bufs=4))
        outp = ctx.enter_context(tc.tile_pool(name="outp", bufs=4))
        ps_st = ctx.enter_context(tc.tile_pool(name="ps_st", bufs=2, space="PSUM"))
        ps_o = ctx.enter_context(tc.tile_pool(name="ps_o", bufs=2, space="PSUM"))

        # ---- PE warmup + input DMA ----
        # zero-input matmuls keep the PE busy from t~7.5us so the HAM clock
        # gate opens (1.2 -> 2.4 GHz, needs ~3us of activity) while the
        # input DMAs stream in.
        scr_sb = const.tile([128, 512], BF16, tag="scr")
        nc.vector.memset(scr_sb[:], 0.0)
        warm_ps = ps_st.tile([128, 1024], F32, tag="st", name="warm")
        for _ in range(8):
            nc.tensor.matmul(warm_ps[:, 0:512], scr_sb[:, 0:128], scr_sb[:])

        wqk_sb = const.tile([128, 2, 4, 256], BF16, tag="wqk")
        wv_sb = const.tile([128, 4, 256], BF16, tag="wv")
        x_sb = const.tile([128, 4, 4, 512], BF16, tag="x")
        # three HWDGE rings in parallel; critical slices first on each
        nc.sync.dma_start(out=wqk_sb[:, 0], in_=wqk_d[:, 0])
        nc.scalar.dma_start(out=x_sb[:, 0], in_=x_d[:, 0])
        nc.gpsimd.dma_start(out=wv_sb[:], in_=wv_d[:])
        nc.sync.dma_start(out=wqk_sb[:, 1], in_=wqk_d[:, 1])
        nc.sync.dma_start(out=x_sb[:, 1], in_=x_d[:, 1])
        nc.gpsimd.dma_start(out=x_sb[:, 2], in_=x_d[:, 2])
        nc.scalar.dma_start(out=x_sb[:, 3], in_=x_d[:, 3])

        ones_sb = const.tile([1, 64], F32, tag="ones")
        nc.vector.memset(ones_sb[:], 1.0)

        q_sb = [qkp.tile([128, L], BF16, tag=f"q{p}", name=f"q{p}") for p in range(2)]
        k_sb = [qkp.tile([128, L], BF16, tag=f"k{p}", name=f"k{p}") for p in range(2)]
        # all vt blocks in one tile (fewer sems); ones column set once
        vt_sb = const.tile([128, NJB, 4, 65], BF16, tag="vt")
        nc.vector.memset(vt_sb[:, :, :, 64:65], 1.0)

        # ---- projection groups (PE work in the proj/bcast PSUM ring) ----
        def g_qk(p, qk, lc):
            # full 512-wide column group of q (qk=0) or k (qk=1), pair p
            def f():
                dst = q_sb[p] if qk == 0 else k_sb[p]
                ps = ps_o.tile([128, 512], F32, tag="proj", name="projg")
                for dc in range(4):
                    nc.tensor.matmul(
                        ps[:],
                        wqk_sb[:, p, dc, qk * 128 : (qk + 1) * 128],
                        x_sb[:, lc, dc, :],
                        start=(dc == 0),
                        stop=(dc == 3),
                    )
                nc.vector.tensor_copy(dst[:, lc * 512 : (lc + 1) * 512], ps[:])

            return f

        def g_qk_pieces(p, qk, lc):
            # the same group chopped into 4 PE pieces (2 matmuls, N=256)
            # + 2 DVE copies, each piece <=~350ns so it fits the per-period
            # PE slack without stalling ScalarE
            dst = q_sb[p] if qk == 0 else k_sb[p]
            state = {}

            def piece(half, dh):
                def f():
                    if "ps" not in state:
                        state["ps"] = ps_o.tile(
                            [128, 512], F32, tag="proj", name="projh"
                        )
                    ps = state["ps"]
                    for dc in (2 * dh, 2 * dh + 1):
                        nc.tensor.matmul(
                            ps[:, half * 256 : (half + 1) * 256],
                            wqk_sb[:, p, dc, qk * 128 : (qk + 1) * 128],
                            x_sb[:, lc, dc, half * 256 : (half + 1) * 256],
                            start=(dc == 0),
                            stop=(dc == 3),
                        )
                    if dh == 1:
                        nc.vector.tensor_copy(
                            dst[:, lc * 512 + half * 256 : lc * 512 + (half + 1) * 256],
                            ps[:, half * 256 : (half + 1) * 256],
                        )

                return f

            return [piece(0, 0), piece(0, 1), piece(1, 0), piece(1, 1)]

        def g_vt(jb):
            def f():
                ps = ps_o.tile([128, 512], F32, tag="proj", name="projv")
                for dc in range(4):
                    nc.tensor.matmul(
                        ps[:, 0:256],
                        x_sb[:, jb // 4, dc, (jb % 4) * 128 : (jb % 4 + 1) * 128],
                        wv_sb[:, dc, :],
                        start=(dc == 0),
                        stop=(dc == 3),
                    )
                nc.vector.tensor_copy(
                    vt_sb[:, jb, :, 0:64],
                    ps[:, 0:256].rearrange("par (h e) -> par h e", e=64),
                )

            return f

        # ---- attention blocks, software-pipelined ----
        def emit_block(p, ic, fillers, lag, norm_carry):
            # one (head-pair, query-chunk) block: 16 exp periods.
            # Per period t: exp(t) | st(t+1) | carried normB | fillers |
            # attnv(t-lag). norm split: normA (DVE den+recip) right after
            # the final attnv; normB (PE bcast + DVE mult + DMA) carried
            # into the next block's first periods.
            fillers = dict(fillers)
            i0 = ic * 512

            def st_mms(jb):
                st = ps_st.tile([128, 1024], F32, tag="st")
                for hp in range(2):
                    nc.tensor.matmul(
                        st[:, hp * 512 : (hp + 1) * 512],
                        k_sb[p][hp * 64 : (hp + 1) * 64, jb * 128 : (jb + 1) * 128],
                        q_sb[p][hp * 64 : (hp + 1) * 64, i0 : i0 + 512],
                        start=True,
                        stop=True,
                    )
                return st

            o_ps = [
                ps_o.tile([65, 512], F32, tag="oacc", name="o_acc") for _ in range(2)
            ]

            def attnv(jb, se_t):
                for hp in range(2):
                    nc.tensor.matmul(
                        o_ps[hp][:],
                        vt_sb[:, jb, 2 * p + hp, :],
                        se_t[:, hp * 512 : (hp + 1) * 512],
                        start=(jb == 0),
                        stop=(jb == NJB - 1),
                    )

            pend = []
            st_cur = st_mms(0)
            for jb in range(NJB):
                se_t = sx.tile([128, 1024], BF16, tag="se")
                nc.scalar.activation(se_t[:], st_cur[:], Exp, scale=SCALE)
                if jb + 1 < NJB:
                    st_cur = st_mms(jb + 1)
                if jb < len(norm_carry):
                    norm_carry[jb]()
                for f in fillers.get(jb, ()):
                    f()
                pend.append((jb, se_t))
                while len(pend) > lag:
                    j, s = pend.pop(0)
                    attnv(j, s)
            while pend:
                j, s = pend.pop(0)
                attnv(j, s)

            # normA now; normB returned for the next block to interleave
            recips = []
            for hp in range(2):
                den_sb = nrm.tile([1, 512], F32, tag="den")
                nc.vector.tensor_copy(den_sb[:], o_ps[hp][64:65, :])
                recip = nrm.tile([1, 512], F32, tag="recip")
                # NB: approx-recip reads garbage from PSUM on HW; SBUF in only
                nc.vector.reciprocal_approx_fast(out=recip[:], in_=den_sb[:])
                recips.append(recip)

            def make_normB(hp):
                def f():
                    hh = 2 * p + hp
                    bc_ps = ps_o.tile([128, 512], F32, tag="proj", name="bcast")
                    nc.tensor.matmul(
                        bc_ps[0:64, :], ones_sb[:], recips[hp][:], start=True, stop=True
                    )
                    rbc = nrm.tile([64, 512], F32, tag="rbc")
                    nc.vector.tensor_copy(rbc[:], bc_ps[0:64, :])
                    ot = outp.tile([64, 512], F32, tag="ot")
                    nc.vector.tensor_mul(ot[:], o_ps[hp][0:64, :], rbc[:])
                    nc.sync.dma_start(
                        out=out_d[hh * 64 : (hh + 1) * 64, i0 : i0 + 512], in_=ot[:]
                    )

                return f

            return [make_normB(0), make_normB(1)]

        # ---- filler schedule ----
        # block 0 (p0,ic0) is demand-bound: it must produce all 15 remaining
        # vt blocks + k0 columns + q0-lc1 while running attention; full
        # groups (less ldweights overhead) front-loaded as hard as the st/
        # attnv pipeline allows. Blocks 1-3 have slack: chopped pieces only.
        fill0 = {
            0: [g_vt(1), g_vt(2), g_vt(3)],
            1: [g_vt(4)],
            2: [g_qk(0, 1, 1)],  # k0 lc1 (before st(4))
            3: [g_vt(5)],
            4: [g_vt(6)],
            5: [g_qk(0, 1, 2)],  # k0 lc2 (before st(8))
            6: [g_vt(7)],
            7: [g_vt(8)],
            8: [g_vt(9)],
            9: [g_qk(0, 1, 3)],  # k0 lc3 (before st(12))
            10: [g_vt(10)],
            11: [g_vt(11)],
            12: [g_vt(12)],
            13: [g_vt(13)],
            14: [g_vt(14), g_vt(15)],
            15: [g_qk(0, 0, 1)],  # q0 lc1 (before block 1)
        }
        # fix first-block vt ordering: g_vt above uses pair-0 k indexing; OK.
        pieces = []
        for p_, qk_, lc_ in [
            (0, 0, 2),  # q0 lc2 (before block 2)
            (1, 0, 0),
            (1, 1, 0),  # q1/k1 lc0
            (0, 0, 3),  # q0 lc3 (before block 3)
            (1, 0, 1),
            (1, 1, 1),
            (1, 0, 2),
            (1, 1, 2),
            (1, 0, 3),
            (1, 1, 3),
        ]:
            pieces.append(g_qk_pieces(p_, qk_, lc_))
        # distribute: blocks 1-3, one piece per period, in dependency order.
        # q0-lc2 must finish inside block 1; q0-lc3 inside block 2; all
        # q1/k1 before block 4.
        fills = [fill0, {}, {}, {}, {}, {}, {}, {}]
        flat = [pc for grp in pieces for pc in grp]  # 40 pieces
        bi, jb = 1, 0
        for pc in flat:
            fills[bi].setdefault(jb, []).append(pc)
            jb += 1
            if jb == NJB:
                bi, jb = bi + 1, 0

        # ---- prologue: minimal path to the first exp ----
        g_qk(0, 0, 0)()  # q0 lc0
        g_qk(0, 1, 0)()  # k0 lc0
        g_vt(0)()

        norm_carry = []
        for bi2, (p_, ic_) in enumerate([(p, ic) for p in range(2) for ic in range(4)]):
            lag = 0 if bi2 == 7 else 1
            norm_carry = emit_block(p_, ic_, fills[bi2], lag, norm_carry)
        for f in norm_carry:
            f()

    nc.finalize()
    return nc


def _get_nc():
    if "nc" not in _COMPILED:
        _COMPILED["nc"] = _build_nc()
    return _COMPILED["nc"]


def _prep_inputs(x, w_qkv):
    """Per-core input maps (host-side sharding)."""
    import ml_dtypes

    bf16 = ml_dtypes.bfloat16
    in_maps = []
    for c in range(N_CORES):
        b, g = c // 2, c % 2
        # x[b] [512, 2048] -> [p, lc, dc, l'] so every DMA descriptor is a
        # 4KB contiguous run
        xb = np.ascontiguousarray(
            x[b].reshape(4, 128, 4, 512).transpose(1, 2, 0, 3)
        ).astype(bf16)
        # w rows for this head group, transposed, pair-major:
        # wqkT[p, pair, dc, 0:128]  = q columns of head-pair `pair`
        # wqkT[p, pair, dc, 128:256] = k columns of head-pair `pair`
        wq_rows = w_qkv[256 * g : 256 * (g + 1), :]  # [256, 512]
        wk_rows = w_qkv[512 + 256 * g : 512 + 256 * (g + 1), :]  # [256, 512]
        wqT = wq_rows.T.reshape(4, 128, 2, 128)  # [dc, p, pair, o]
        wkT = wk_rows.T.reshape(4, 128, 2, 128)
        wqkT = np.ascontiguousarray(
            np.concatenate([wqT, wkT], axis=3).transpose(1, 2, 0, 3)
        ).astype(bf16)  # [p, pair, dc, 256]
        wv_rows = w_qkv[1024 + 256 * g : 1024 + 256 * (g + 1), :]  # [256, 512]
        wvT = np.ascontiguousarray(
            wv_rows.T.reshape(4, 128, 256).transpose(1, 0, 2)
        ).astype(bf16)
        in_maps.append({"x": xb, "wqkT": wqkT, "wvT": wvT})
    return in_maps


def kernel(x, w_qkv):
    global LAST_RESULTS
    from concourse.bass_utils import run_bass_kernel_spmd

    nc = _get_nc()
    in_maps = _prep_inputs(np.asarray(x), np.asarray(w_qkv))
    res = run_bass_kernel_spmd(
        nc, in_maps, core_ids=list(range(N_CORES)), trace=TRACE
    )
    LAST_RESULTS = res
    out = np.empty((B, D, L), dtype=np.float32)
    for c in range(N_CORES):
        b, g = c // 2, c % 2
        out[b, 256 * g : 256 * (g + 1), :] = res.results[c]["out"]
    return out
